# revision 1
# baseline (speedup 1.0000x reference)
"""EMA scan kernel for Trainium2 (8 NeuronCores, data-parallel over batch).

y[n] = w*x[n] + (1-w)*y[n-1],  y[-1] = initial_state

Full input (16, 8, 256, 2048) f32 is sharded 2 batches per core; each core
runs the recurrence with the DVE tensor_tensor_scan instruction on
[128 channels x 2048 frames] tiles (channels on partitions, frames on the
free axis). The (8, 256) weight is replicated as per-partition scalar
columns.
"""

import numpy as np

import concourse.bacc as bacc
import concourse.mybir as mybir
from concourse.bass_utils import run_bass_kernel_spmd
from concourse.tile import TileContext

BATCH, N_RES, N_BINS, N_FRAMES = 16, 8, 256, 2048
N_CORES = 8
B_PER_CORE = BATCH // N_CORES                      # 2
CH_PER_CORE = B_PER_CORE * N_RES * N_BINS          # 4096
N_TILES = CH_PER_CORE // 128                       # 32

_CACHED_NC = {}


def _build(repeat=1, compile=True):
    nc = bacc.Bacc(
        "TRN2", target_bir_lowering=False, debug=False, num_devices=N_CORES
    )
    x = nc.dram_tensor(
        "x", (CH_PER_CORE, N_FRAMES), mybir.dt.float32, kind="ExternalInput"
    )
    wcol = nc.dram_tensor(
        "wcol", (128, N_TILES), mybir.dt.float32, kind="ExternalInput"
    )
    acol = nc.dram_tensor(
        "acol", (128, N_TILES), mybir.dt.float32, kind="ExternalInput"
    )
    init = nc.dram_tensor(
        "init", (128, N_TILES), mybir.dt.float32, kind="ExternalInput"
    )
    y = nc.dram_tensor(
        "y", (CH_PER_CORE, N_FRAMES), mybir.dt.float32, kind="ExternalOutput"
    )
    xa, ya = x.ap(), y.ap()

    with TileContext(nc) as tc:
        with tc.tile_pool(name="const", bufs=1) as cpool, tc.tile_pool(
            name="xin", bufs=11
        ) as xpool, tc.tile_pool(name="work", bufs=9) as pool:
            wt = cpool.tile([128, N_TILES], mybir.dt.float32)
            at = cpool.tile([128, N_TILES], mybir.dt.float32)
            it = cpool.tile([128, N_TILES], mybir.dt.float32)
            # scan-side consts first on SP (tiny, land before the first x
            # sliver); the scale const on the ACT queue it is used from
            nc.sync.dma_start(out=at[:], in_=acol.ap())
            nc.sync.dma_start(out=it[:], in_=init.ap())
            nc.scalar.dma_start(out=wt[:], in_=wcol.ap())

            def emit_tile(j, splits):
                rows = slice(j * 128, (j + 1) * 128)
                prev_tail = None
                c0 = 0
                for clen in splits:
                    cols = slice(c0, c0 + clen)
                    c0 += clen
                    xt = xpool.tile([128, clen], mybir.dt.float32)
                    nc.sync.dma_start(out=xt[:], in_=xa[rows, cols])
                    st = pool.tile([128, clen], mybir.dt.float32)
                    # st = x * w  (per-partition scalar) on ScalarE
                    nc.scalar.activation(
                        st[:],
                        xt[:],
                        mybir.ActivationFunctionType.Copy,
                        scale=wt[:, j : j + 1],
                    )
                    # y[t] = a*y[t-1] + st[t] on DVE, in place; chunks chain
                    # through the previous chunk's last column
                    nc.vector.tensor_tensor_scan(
                        st[:],
                        at[:, j : j + 1].to_broadcast((128, clen)),
                        st[:],
                        initial=it[:, j : j + 1] if prev_tail is None else prev_tail,
                        op0=mybir.AluOpType.mult,
                        op1=mybir.AluOpType.add,
                    )
                    prev_tail = st[:, clen - 1 : clen]
                    # store via the idle GpSimd SWDGE queue: its wait on the
                    # scan must not block load issue (SP) or the scales (ACT)
                    nc.gpsimd.dma_start(out=ya[rows, cols], in_=st[:])

            for j in _rep(range(N_TILES), repeat):
                # chunk the pipeline-fill tile so the first scan starts as
                # soon as a small sliver has landed, and the tail tile so
                # its final store is short and overlaps the preceding scan
                emit_tile(
                    j,
                    {
                        0: (512, 512, 512, 512),
                        N_TILES - 1: (1024, 1024),
                    }.get(j, (N_FRAMES,)),
                )
    if compile:
        nc.compile()
    return nc


def _rep(it, n):
    for _ in range(n):
        yield from it


def _get_nc(repeat=1):
    if repeat not in _CACHED_NC:
        _CACHED_NC[repeat] = _build(repeat)
    return _CACHED_NC[repeat]


def _run(input, initial_state, weight, trace=False, repeat=1):
    input = np.ascontiguousarray(np.asarray(input, dtype=np.float32))
    initial_state = np.asarray(initial_state, dtype=np.float32)
    weight = np.asarray(weight, dtype=np.float32)

    w_flat = np.clip(weight, 0.0, 1.0).reshape(-1)            # (2048,)
    w_ch = np.tile(w_flat, B_PER_CORE)                        # (4096,) per core
    wcol = np.ascontiguousarray(w_ch.reshape(N_TILES, 128).T)
    acol = np.ascontiguousarray((1.0 - w_ch).reshape(N_TILES, 128).T)

    in_maps = []
    for k in range(N_CORES):
        xk = input[k * B_PER_CORE : (k + 1) * B_PER_CORE].reshape(
            CH_PER_CORE, N_FRAMES
        )
        ik = initial_state[k * B_PER_CORE : (k + 1) * B_PER_CORE].reshape(
            CH_PER_CORE
        )
        in_maps.append(
            {
                "x": np.ascontiguousarray(xk),
                "wcol": wcol,
                "acol": acol,
                "init": np.ascontiguousarray(ik.reshape(N_TILES, 128).T),
            }
        )

    res = run_bass_kernel_spmd(
        _get_nc(repeat), in_maps, core_ids=list(range(N_CORES)), trace=trace
    )
    out = np.empty((BATCH, N_RES, N_BINS, N_FRAMES), dtype=np.float32)
    for k in range(N_CORES):
        out[k * B_PER_CORE : (k + 1) * B_PER_CORE] = np.asarray(
            res.results[k]["y"]
        ).reshape(B_PER_CORE, N_RES, N_BINS, N_FRAMES)
    return out, res


def kernel(input, initial_state, weight):
    out, _ = _run(input, initial_state, weight, trace=False)
    return out

